# revision 8
# baseline (speedup 1.0000x reference)
"""Trainium2 Bass kernel for nn_BottomLevelDecoderRNN.

Strategy (8 NeuronCores, model-parallel over the hidden dim):
  - The 256 timesteps (16 segments x 16 steps) are strictly sequential, and the
    full weight set (77.5MB fp32) cannot stay SBUF-resident, so data-parallel
    batch sharding would re-stream weights from HBM every step. Instead each
    core owns a 128-row slice of the hidden dim (512 of the 4096 gate rows of
    each cell), keeps its weight shard SBUF-resident in bf16, and streams the
    full batch (N=256) as the matmul moving operand.
  - States live transposed: [feature -> partitions, batch -> free].
  - prev only enters cell1 through W_p1 @ prev and prev = W_out @ h2 + b_out,
    so W_comb = W_p1 @ W_out is folded on the host: the output projection
    leaves the recurrence entirely. Per step only two 64KB AllGathers (h1, h2
    slices, bf16) remain, overlapped with matmuls.
  - The embed contribution + all biases fold into a per-segment E1 tensor and
    the segment init states, both precomputed on the host (one-time, tiny
    fraction of total FLOPs).
  - The out projection runs gate-row-sharded (64 rows/core) inside the loop,
    off the critical path; host assembles [B, T, OUT].
"""

import os
import sys
import types

import numpy as np
import ml_dtypes

import concourse.bass as bass
import concourse.mybir as mybir
import concourse.tile as tile
from concourse.bass_utils import run_bass_kernel_spmd

BF = mybir.dt.bfloat16
F32 = mybir.dt.float32
AF = mybir.ActivationFunctionType
ALU = mybir.AluOpType

N_CORES = 8
B = 256
H = 1024
COND = 512
OUT = 512
STEPS = 16
KH = 8          # k-tiles over H
MT = 4          # gate m-tiles per core (512 gate rows)

LAST_RESULTS = None  # stashed BassKernelResults for the dev harness


# ---------------------------------------------------------------------------
# walrus in this container accepts only ONE embedded sync-wait per
# instruction; hoist extras onto preceding NoOps on the same engine
# (engines execute in order, so waiting earlier is safe).
_WS_CTR = [0]


def _split_excess_waits(nc, max_waits=1):
    for f in nc.m.functions:
        for blk in f.blocks:
            out = []
            for ins in blk.instructions:
                si = ins.sync_info
                if si is not None and si.on_wait is not None and len(si.on_wait) > max_waits:
                    waits = list(si.on_wait)
                    keep = waits[-max_waits:]
                    hoist = waits[:-max_waits]
                    for i in range(0, len(hoist), max_waits):
                        _WS_CTR[0] += 1
                        nop = mybir.InstNoOp(
                            name=f"I-waitsplit-{_WS_CTR[0]}",
                            engine=ins.engine,
                            ins=[], outs=[],
                            sync_info=mybir.SyncInfo(
                                on_wait=hoist[i:i + max_waits], on_update=[]),
                        )
                        nc.register_instruction(nop, overwrite=True)
                        out.append(nop)
                    si.on_wait = keep
                out.append(ins)
            blk.instructions[:] = out


def _install_ntff_hook():
    """antenv.axon_hooks is missing in this image; synthesize it so
    trace=True works (used by the dev harness only)."""
    if 'antenv.axon_hooks' in sys.modules:
        return
    try:
        import antenv
        mod = types.ModuleType("antenv.axon_hooks")
        mod._hook = None
        mod.set_axon_ntff_profile_hook = lambda h: setattr(mod, '_hook', h)
        mod.get_axon_ntff_profile_hook = lambda: mod._hook
        sys.modules['antenv.axon_hooks'] = mod
        antenv.axon_hooks = mod
        from trn_agent_boot.trn_boot import _ntff_profile_via_ctypes
        mod.set_axon_ntff_profile_hook(
            _ntff_profile_via_ctypes('/opt/axon/libaxon_pjrt.so'))
    except Exception:
        pass


def _wtiles(wt):
    """[K, M_tot] fp32 lhsT matrix -> [128, (K/128)*(M_tot/128)*128] bf16 with
    free index ((k*mt + m)*128 + j)."""
    K, M_tot = wt.shape
    kt = K // 128
    if M_tot < 128:
        arr = wt.reshape(kt, 128, M_tot).transpose(1, 0, 2).reshape(128, kt * M_tot)
        return arr.astype(ml_dtypes.bfloat16)
    mt = M_tot // 128
    arr = wt.reshape(kt, 128, mt, 128).transpose(1, 0, 2, 3).reshape(128, kt * mt * 128)
    return arr.astype(ml_dtypes.bfloat16)


def _pfirst(x, kt):
    """[K, N] -> [128, (K/128)*N] partition-first (p, k*N+n)."""
    K, N = x.shape
    return x.reshape(kt, 128, N).transpose(1, 0, 2).reshape(128, kt * N)


def _build(n_seg):
    nc = bass.Bass()
    T = n_seg * STEPS

    # per-core inputs (weights differ per core; program is identical)
    WH1 = nc.dram_tensor("WH1", [128, KH * MT * 128], BF, kind="ExternalInput")
    WCB = nc.dram_tensor("WCB", [128, KH * MT * 128], BF, kind="ExternalInput")
    WX2 = nc.dram_tensor("WX2", [128, KH * MT * 128], BF, kind="ExternalInput")
    WH2 = nc.dram_tensor("WH2", [128, KH * MT * 128], BF, kind="ExternalInput")
    WOS = nc.dram_tensor("WOS", [128, KH * 64], BF, kind="ExternalInput")
    BOS = nc.dram_tensor("BOS", [64, 1], F32, kind="ExternalInput")
    B2S = nc.dram_tensor("B2S", [128, MT], F32, kind="ExternalInput")
    E1I = nc.dram_tensor("E1I", [n_seg + 1, 128, MT * 256], F32, kind="ExternalInput")
    H1I = nc.dram_tensor("H1I", [n_seg, 128, KH * 256], BF, kind="ExternalInput")
    H2I = nc.dram_tensor("H2I", [n_seg, 128, KH * 256], BF, kind="ExternalInput")
    C1I = nc.dram_tensor("C1I", [n_seg, 128, 256], F32, kind="ExternalInput")
    C2I = nc.dram_tensor("C2I", [n_seg, 128, 256], F32, kind="ExternalInput")
    OUTC = nc.dram_tensor("OUTC", [T, 64, 256], F32, kind="ExternalOutput")

    rg = [list(range(N_CORES))]

    with tile.TileContext(nc) as tc:
        with tc.tile_pool(name="const", bufs=1) as cpool, \
             tc.tile_pool(name="seg", bufs=2) as spool, \
             tc.tile_pool(name="state", bufs=3) as stp, \
             tc.tile_pool(name="work", bufs=3) as wp, \
             tc.tile_pool(name="psum", bufs=1, space="PSUM") as pp, \
             tc.tile_pool(name="dramio", bufs=2, space="DRAM") as dio, \
             tc.tile_pool(name="dramag", bufs=1, space="DRAM") as dag:

            wh1 = cpool.tile([128, KH * MT * 128], BF)
            wcb = cpool.tile([128, KH * MT * 128], BF)
            wx2 = cpool.tile([128, KH * MT * 128], BF)
            wh2 = cpool.tile([128, KH * MT * 128], BF)
            wos = cpool.tile([128, KH * 64], BF)
            bos = cpool.tile([64, 1], F32)
            b2s = cpool.tile([128, MT], F32)
            for tl, src in ((wh1, WH1), (wcb, WCB), (wx2, WX2), (wh2, WH2),
                            (wos, WOS), (bos, BOS), (b2s, B2S)):
                nc.sync.dma_start(tl[:], src[:])

            zer = cpool.tile([128, KH * 256], BF)
            nc.vector.memset(zer[:], 0.0)

            psum1 = pp.tile([128, MT, 512], F32)   # 4 banks, cell1 (+ out proj)
            psum2 = pp.tile([128, MT, 512], F32)   # 4 banks, cell2

            def wsl(w, k, m):
                return w[:, (k * MT + m) * 128:(k * MT + m) * 128 + 128]

            h2f_carry = zer          # h2n(t-1) for the W_comb path; 0 at t=0
            out_pending = None       # (t, h2f tile) for the deferred out proj
            for s in range(n_seg):
                e1t = spool.tile([128, MT * 256], F32, tag="e1")
                nc.sync.dma_start(e1t[:], E1I[s])
                if s == 0:
                    e1t_first = spool.tile([128, MT * 256], F32, tag="e1f")
                    nc.sync.dma_start(e1t_first[:], E1I[n_seg])
                h1f_cur = stp.tile([128, KH * 256], BF, tag="h1f")
                nc.sync.dma_start(h1f_cur[:], H1I[s])
                h2st_cur = stp.tile([128, KH * 256], BF, tag="h2st")
                nc.sync.dma_start(h2st_cur[:], H2I[s])
                c1_cur = stp.tile([128, 256], F32, tag="c1")
                nc.sync.dma_start(c1_cur[:], C1I[s])
                c2_cur = stp.tile([128, 256], F32, tag="c2")
                nc.sync.dma_start(c2_cur[:], C2I[s])

                for ts in range(STEPS):
                    t = s * STEPS + ts
                    e1_use = e1t_first if t == 0 else e1t

                    # ---- cell1 matmuls: all WH1 first (h1f arrives one AG
                    # earlier than h2f), then WCB k-outer so the gather DMA
                    # chunks are consumed in arrival order.
                    for m in range(MT):
                        for k in range(KH):
                            nc.tensor.matmul(
                                psum1[:, m, 0:256], wsl(wh1, k, m),
                                h1f_cur[:, k * 256:(k + 1) * 256],
                                start=(k == 0), stop=False)
                    for k in range(KH):
                        for m in range(MT):
                            nc.tensor.matmul(
                                psum1[:, m, 0:256], wsl(wcb, k, m),
                                h2f_carry[:, k * 256:(k + 1) * 256],
                                start=False, stop=(k == KH - 1))

                    # deferred out projection of step t-1: needs only
                    # h2f(t-1); runs on PE inside the AG(h1,t) window.
                    if out_pending is not None:
                        pt, ph2 = out_pending
                        for k in range(KH):
                            nc.tensor.matmul(
                                psum2[0:64, 0, 256:512], wos[:, k * 64:(k + 1) * 64],
                                ph2[:, k * 256:(k + 1) * 256],
                                start=(k == 0), stop=(k == KH - 1))
                        outsb = wp.tile([64, 256], F32, tag="outsb")
                        nc.scalar.activation(outsb[:], psum2[0:64, 0, 256:512],
                                             AF.Identity, bias=bos[:, 0:1])
                        nc.sync.dma_start(OUTC[pt], outsb[:])
                        out_pending = None

                    # ---- cell1 gates
                    pre1 = wp.tile([128, MT * 256], F32, tag="pre1")
                    nc.vector.tensor_tensor(
                        pre1[:].rearrange("p (m n) -> p m n", m=MT),
                        psum1[:, :, 0:256],
                        e1_use[:].rearrange("p (m n) -> p m n", m=MT),
                        ALU.add)
                    g1a = wp.tile([128, MT * 256], F32, tag="g1a")
                    nc.scalar.activation(g1a[:, 0:512], pre1[:, 0:512], AF.Sigmoid)
                    nc.scalar.activation(g1a[:, 512:768], pre1[:, 512:768], AF.Tanh)
                    nc.scalar.activation(g1a[:, 768:1024], pre1[:, 768:1024], AF.Sigmoid)
                    tmp1 = wp.tile([128, 256], F32, tag="tmp1")
                    nc.vector.tensor_tensor(tmp1[:], g1a[:, 0:256], g1a[:, 512:768], ALU.mult)
                    c1n = stp.tile([128, 256], F32, tag="c1")
                    nc.vector.tensor_tensor(c1n[:], g1a[:, 256:512], c1_cur[:], ALU.mult)
                    nc.vector.tensor_tensor(c1n[:], c1n[:], tmp1[:], ALU.add)
                    tct1 = wp.tile([128, 256], F32, tag="tct1")
                    nc.scalar.activation(tct1[:], c1n[:], AF.Tanh)
                    h1s = wp.tile([128, 256], BF, tag="h1s")
                    nc.vector.tensor_tensor(h1s[:], g1a[:, 768:1024], tct1[:], ALU.mult)
                    c1_cur = c1n

                    # ---- AllGather h1n
                    agi1 = dio.tile([128, 256], BF, tag="agi1")
                    nc.sync.dma_start(agi1[:], h1s[:])
                    ago1 = dag.tile([KH, 128, 256], BF, addr_space="Shared",
                                    tag=f"ago1_{t}", name=f"ago1_{t}")
                    nc.gpsimd.collective_compute(
                        "AllGather", ALU.bypass, replica_groups=rg,
                        ins=[agi1.opt()], outs=[ago1.opt()])
                    h1f_next = stp.tile([128, KH * 256], BF, tag="h1f")
                    for half in range(2):
                        kk = KH // 2
                        nc.sync.dma_start(
                            h1f_next[:, half * kk * 256:(half + 1) * kk * 256]
                            .rearrange("p (k n) -> p k n", k=kk),
                            ago1[half * kk:(half + 1) * kk].rearrange("k p n -> p k n"))

                    # ---- cell2 matmuls: all WH2 first (overlaps AG), then
                    # WX2 k-outer consuming the gathered h1n in arrival order.
                    for m in range(MT):
                        for k in range(KH):
                            nc.tensor.matmul(
                                psum2[:, m, 0:256], wsl(wh2, k, m),
                                h2st_cur[:, k * 256:(k + 1) * 256],
                                start=(k == 0), stop=False)
                    for k in range(KH):
                        for m in range(MT):
                            nc.tensor.matmul(
                                psum2[:, m, 0:256], wsl(wx2, k, m),
                                h1f_next[:, k * 256:(k + 1) * 256],
                                start=False, stop=(k == KH - 1))

                    # ---- cell2 gates (bias via ACT per gate tile)
                    g2a = wp.tile([128, MT * 256], F32, tag="g2a")
                    for m, fn in ((0, AF.Sigmoid), (1, AF.Sigmoid), (2, AF.Tanh), (3, AF.Sigmoid)):
                        nc.scalar.activation(
                            g2a[:, m * 256:(m + 1) * 256], psum2[:, m, 0:256],
                            fn, bias=b2s[:, m:m + 1])
                    tmp2 = wp.tile([128, 256], F32, tag="tmp2")
                    nc.vector.tensor_tensor(tmp2[:], g2a[:, 0:256], g2a[:, 512:768], ALU.mult)
                    c2n = stp.tile([128, 256], F32, tag="c2")
                    nc.vector.tensor_tensor(c2n[:], g2a[:, 256:512], c2_cur[:], ALU.mult)
                    nc.vector.tensor_tensor(c2n[:], c2n[:], tmp2[:], ALU.add)
                    tct2 = wp.tile([128, 256], F32, tag="tct2")
                    nc.scalar.activation(tct2[:], c2n[:], AF.Tanh)
                    h2s = wp.tile([128, 256], BF, tag="h2s")
                    nc.vector.tensor_tensor(h2s[:], g2a[:, 768:1024], tct2[:], ALU.mult)
                    c2_cur = c2n

                    # ---- AllGather h2n
                    agi2 = dio.tile([128, 256], BF, tag="agi2")
                    nc.sync.dma_start(agi2[:], h2s[:])
                    ago2 = dag.tile([KH, 128, 256], BF, addr_space="Shared",
                                    tag=f"ago2_{t}", name=f"ago2_{t}")
                    nc.gpsimd.collective_compute(
                        "AllGather", ALU.bypass, replica_groups=rg,
                        ins=[agi2.opt()], outs=[ago2.opt()])
                    h2f_next = stp.tile([128, KH * 256], BF, tag="h2st")
                    for half in range(2):
                        kk = KH // 2
                        nc.sync.dma_start(
                            h2f_next[:, half * kk * 256:(half + 1) * kk * 256]
                            .rearrange("p (k n) -> p k n", k=kk),
                            ago2[half * kk:(half + 1) * kk].rearrange("k p n -> p k n"))

                    out_pending = (t, h2f_next)
                    h1f_cur = h1f_next
                    h2f_carry = h2f_next
                    h2st_cur = h2f_next

            # final step's out projection
            pt, ph2 = out_pending
            for k in range(KH):
                nc.tensor.matmul(
                    psum2[0:64, 0, 256:512], wos[:, k * 64:(k + 1) * 64],
                    ph2[:, k * 256:(k + 1) * 256],
                    start=(k == 0), stop=(k == KH - 1))
            outsb = wp.tile([64, 256], F32, tag="outsb")
            nc.scalar.activation(outsb[:], psum2[0:64, 0, 256:512],
                                 AF.Identity, bias=bos[:, 0:1])
            nc.sync.dma_start(OUTC[pt], outsb[:])

    _split_excess_waits(nc, max_waits=1)
    return nc


def _host_prep(inputs, n_seg):
    f32 = np.float32
    c = np.asarray(inputs["c"], f32)[:n_seg]
    W_init = np.asarray(inputs["W_init"], f32)
    b_init = np.asarray(inputs["b_init"], f32)
    W_ih1 = np.asarray(inputs["W_ih1"], f32)
    W_hh1 = np.asarray(inputs["W_hh1"], f32)
    b_ih1 = np.asarray(inputs["b_ih1"], f32)
    b_hh1 = np.asarray(inputs["b_hh1"], f32)
    W_ih2 = np.asarray(inputs["W_ih2"], f32)
    W_hh2 = np.asarray(inputs["W_hh2"], f32)
    b_ih2 = np.asarray(inputs["b_ih2"], f32)
    b_hh2 = np.asarray(inputs["b_hh2"], f32)
    W_out = np.asarray(inputs["W_out"], f32)
    b_out = np.asarray(inputs["b_out"], f32)

    # segment init states, replicated host-side (one-time, ~2% of FLOPs)
    emb = c.transpose(0, 2, 1)                      # [S, 512, 256]
    tinit = np.tanh(np.einsum('gk,skn->sgn', W_init, emb) + b_init[None, :, None])

    in_maps = []
    for cid in range(N_CORES):
        sel = np.arange(128 * cid, 128 * cid + 128)
        rows = np.concatenate([g * 1024 + sel for g in range(4)])
        Wc1 = W_ih1[rows, :COND]
        Wp1 = W_ih1[rows, COND:]
        Wcomb = Wp1 @ W_out
        b1 = (b_ih1 + b_hh1)[rows]
        b1full = b1 + Wp1 @ b_out
        b2 = (b_ih2 + b_hh2)[rows]

        e1 = np.einsum('gk,skn->sgn', Wc1, emb) + b1full[None, :, None]
        e1_first = e1[0] - (Wp1 @ b_out)[:, None]
        e1_all = np.concatenate([e1, e1_first[None]], 0)        # [S+1, 512, 256]
        e1_dev = e1_all.reshape(n_seg + 1, 4, 128, 256).transpose(0, 2, 1, 3) \
                       .reshape(n_seg + 1, 128, MT * 256).astype(f32)

        bf = ml_dtypes.bfloat16
        m = {
            "WH1": _wtiles(W_hh1[rows].T),
            "WCB": _wtiles(Wcomb.T),
            "WX2": _wtiles(W_ih2[rows].T),
            "WH2": _wtiles(W_hh2[rows].T),
            "WOS": _wtiles(W_out[64 * cid:64 * cid + 64].T),
            "BOS": b_out[64 * cid:64 * cid + 64].reshape(64, 1).astype(f32),
            "B2S": b2.reshape(4, 128).T.copy().astype(f32),
            "E1I": e1_dev,
            "H1I": np.stack([_pfirst(tinit[s, 0:1024], KH) for s in range(n_seg)]).astype(bf),
            "H2I": np.stack([_pfirst(tinit[s, 1024:2048], KH) for s in range(n_seg)]).astype(bf),
            "C1I": tinit[:, 2048 + 128 * cid:2048 + 128 * cid + 128].astype(f32),
            "C2I": tinit[:, 3072 + 128 * cid:3072 + 128 * cid + 128].astype(f32),
        }
        in_maps.append(m)
    return in_maps


def kernel(**inputs):
    global LAST_RESULTS
    n_seg = int(os.environ.get("BASS_LSTM_NSEG", "0")) or np.asarray(inputs["c"]).shape[0]
    assert int(inputs["length"]) == 2 * STEPS

    trace = os.environ.get("BASS_LSTM_TRACE", "") == "1"
    if trace:
        _install_ntff_hook()

    nc = _build(n_seg)
    in_maps = _host_prep(inputs, n_seg)
    res = run_bass_kernel_spmd(nc, in_maps, core_ids=list(range(N_CORES)),
                               trace=trace)
    LAST_RESULTS = res

    T = n_seg * STEPS
    # OUTC per core: [T, 64, 256] -> concat over cores on the 64-row axis
    full = np.concatenate([r["OUTC"] for r in res.results], axis=1)  # [T, 512, 256]
    outputs = np.ascontiguousarray(full.transpose(2, 0, 1))          # [B, T, OUT]

    k = int(inputs["k"]); epoch = int(inputs["epoch"])
    ratio = np.float32(k / (k + np.exp(epoch / k)))
    return outputs, ratio
